# revision 7
# baseline (speedup 1.0000x reference)
"""CostVolumeLayer3D Trainium2 kernel.

Computes cv[b, ch, d, y, x] = (1/125) * sum_c x1[b,c,d,y,x] * x2[b,c,d-h,y-i,x-j]
for the 45 channels that survive the reference's channel-collapse
(ch = (5*(i+j)+h) % 125, last write in (i,j,h) loop order wins -> for each
diagonal s=i+j the winner is i=min(2,s+2), j=s-i). Remaining 80 channels are 0.

Sharding: depth D=32 split across 8 cores (4 output slices each); the host
supplies each core a zero/halo-padded x2 shard so every shifted window is a
plain strided view.

Per-core layout: SBUF partitions = (b, c) = 2*64 = 128. Free axis = padded
(d', y', x') volume of x2, so a 3D shift is a free-axis offset view.
DVE computes shifted elementwise products (fp16, 2x mode); PE reduces over
the 64 channels via one-hot fp16 matmuls accumulating all 45 shifts into
PSUM rows 0..89 = (shift, b); ACT extracts with the 1/125 scale to fp32.
"""

import numpy as np

_B, _C, _D, _H, _W = 2, 64, 32, 64, 64
_R = 2
_NCH = 125
_NCORES = 8
_DL = _D // _NCORES          # output depth slices per core (4)
_DH = _DL + 2 * _R           # x2 depth slices incl. halo (8)
_YB = 32                     # y block
_NYB = _H // _YB             # 2
_YH = _YB + 2 * _R           # 36
_XH = _W + 2 * _R            # 68
_COMPUTE_DT = "float16"      # on-device product dtype
_NFREE = _YB * _W            # free elems per tile (2048)
_MMN = 512                   # matmul moving free dim


def _shift_table():
    """45 surviving shifts as (out_channel, d_off, y_off, x_off) where the
    x2 window for output (t, y, x) starts at padded index
    (t + d_off, y + y_off, x + x_off)."""
    shifts = []
    for sd in range(-4, 5):
        i = min(2, sd + 2)
        j = sd - i
        for h in range(-2, 3):
            shifts.append(((5 * sd + h) % _NCH, _R - h, _R - i, _R - j))
    return shifts


_SHIFTS = _shift_table()
_NS = len(_SHIFTS)           # 45
_NG = 3                      # psum partition groups (32-aligned tile_position)
_GS = _NS // _NG             # shifts per group (15)
_GM = 2 * _GS                # matmul M / psum rows per group (30)
# shifts whose product TT runs on GPSIMD instead of DVE (load balance)
_GPSIMD_SHIFTS = frozenset(s for s in range(_NS) if s % 5 == 2)


def _ones_lhst(np_dt):
    """One matmul weight matrix per shift: within its 32-aligned psum group,
    lhsT[k, s, m] routes partition half k//64 (= batch) of shift s's products
    to group row 2*(s%15) + k//64."""
    a = np.zeros((128, _NS, _GM), dtype=np_dt)
    for s in range(_NS):
        q = s % _GS
        a[0:64, s, 2 * q] = 1.0
        a[64:128, s, 2 * q + 1] = 1.0
    return a


_prog = None


def _build_program():
    global _prog
    if _prog is not None:
        return _prog
    from contextlib import ExitStack

    import concourse.bacc as bacc
    import concourse.mybir as mybir
    import concourse.tile as tile

    dt_in = getattr(mybir.dt, _COMPUTE_DT)
    f32 = mybir.dt.float32
    nc = bacc.Bacc(trn_type="TRN2", debug=False)
    x1_d = nc.dram_tensor("x1", [_B, _C, _DL, _H, _W], dt_in, kind="ExternalInput")
    x2_d = nc.dram_tensor(
        "x2", [_B, _C, _DH, _H + 2 * _R, _XH], dt_in, kind="ExternalInput"
    )
    on_d = nc.dram_tensor("ones", [128, _NS, _GM], dt_in, kind="ExternalInput")
    out_d = nc.dram_tensor("out", [_NS, _B, _DL, _H, _W], f32, kind="ExternalOutput")

    with tile.TileContext(nc) as tc:
        with ExitStack() as ctx:
            constp = ctx.enter_context(tc.tile_pool(name="const", bufs=1))
            x2p = ctx.enter_context(tc.tile_pool(name="x2res", bufs=1))
            x2op = ctx.enter_context(tc.tile_pool(name="x2odd", bufs=1))
            x1p = ctx.enter_context(tc.tile_pool(name="x1", bufs=3))
            prodp = ctx.enter_context(tc.tile_pool(name="prod", bufs=4))
            psump = ctx.enter_context(tc.tile_pool(name="psum", bufs=2, space="PSUM"))
            stagep = ctx.enter_context(tc.tile_pool(name="stage", bufs=2))

            ones_t = constp.tile([128, _NS, _GM], dt_in)
            nc.sync.dma_start(ones_t[:], on_d.ap())

            for yh in range(_NYB):
                y0 = yh * _YB
                x2_t = x2p.tile([128, _DH, _YH, _XH], dt_in)
                nc.sync.dma_start(
                    x2_t[:],
                    x2_d.ap()[:, :, :, y0 : y0 + _YH, :].rearrange(
                        "b c d y x -> (b c) d y x"
                    ),
                )
                # x-shift-by-one copy keeps odd-j windows 4B-aligned (DVE 2x mode)
                x2o_t = x2op.tile([128, _DH, _YH, _XH], dt_in)
                nc.scalar.copy(x2o_t[:, :, :, 0 : _XH - 1], x2_t[:, :, :, 1:_XH])
                for t in range(_DL):
                    x1_t = x1p.tile([128, _YB, _W], dt_in)
                    nc.sync.dma_start(
                        x1_t[:],
                        x1_d.ap()[:, :, t, y0 : y0 + _YB, :].rearrange(
                            "b c y x -> (b c) y x"
                        ),
                    )
                    ps = psump.tile([128, _NFREE], f32)
                    for s, (_ch, dd0, yy0, xx0) in enumerate(_SHIFTS):
                        g, q = divmod(s, _GS)
                        pr = prodp.tile([128, _YB, _W], dt_in)
                        dv = t + dd0
                        if xx0 % 2 == 0:
                            xv = x2_t[:, dv, yy0 : yy0 + _YB, xx0 : xx0 + _W]
                        else:
                            xv = x2o_t[:, dv, yy0 : yy0 + _YB, xx0 - 1 : xx0 - 1 + _W]
                        eng = nc.gpsimd if s in _GPSIMD_SHIFTS else nc.vector
                        eng.tensor_mul(pr[:], x1_t[:], xv)
                        prf = pr[:].rearrange("p y x -> p (y x)")
                        for n in range(_NFREE // _MMN):
                            nc.tensor.matmul(
                                ps[32 * g : 32 * g + _GM, _MMN * n : _MMN * (n + 1)],
                                lhsT=ones_t[:, s, :],
                                rhs=prf[:, _MMN * n : _MMN * (n + 1)],
                                start=(q == 0),
                                stop=(q == _GS - 1),
                            )
                    st = stagep.tile([128, _NFREE], f32)
                    for g in range(_NG):
                        nc.scalar.mul(
                            st[32 * g : 32 * g + _GM, :],
                            ps[32 * g : 32 * g + _GM, :],
                            1.0 / _NCH,
                        )
                        nc.sync.dma_start(
                            out_d.ap()[
                                _GS * g : _GS * (g + 1), :, t, y0 : y0 + _YB, :
                            ].rearrange("s b y x -> (s b) (y x)"),
                            st[32 * g : 32 * g + _GM, :],
                        )
    nc.compile()
    _prog = nc
    return nc


def _np_dt():
    return np.float16 if _COMPUTE_DT == "float16" else np.float32


def _shard_inputs(x1, x2):
    np_dt = _np_dt()
    x2pad = np.pad(
        np.asarray(x2), ((0, 0), (0, 0), (_R, _R), (_R, _R), (_R, _R))
    ).astype(np_dt)
    x1 = np.asarray(x1)
    ones_np = _ones_lhst(np_dt)
    in_maps = []
    for k in range(_NCORES):
        d0 = k * _DL
        in_maps.append(
            {
                "x1": np.ascontiguousarray(x1[:, :, d0 : d0 + _DL].astype(np_dt)),
                "x2": np.ascontiguousarray(x2pad[:, :, d0 : d0 + _DH]),
                "ones": ones_np,
            }
        )
    return in_maps


def _gather(results):
    out = np.zeros((_B, _NCH, _D, _H, _W), dtype=np.float32)
    for k in range(_NCORES):
        o = results[k]["out"]  # [45, B, DL, H, W] fp32
        d0 = k * _DL
        for s, (ch, _dd0, _yy0, _xx0) in enumerate(_SHIFTS):
            out[:, ch, d0 : d0 + _DL] = o[s]
    return out


def _run(in_maps, **kwargs):
    from concourse.bass_utils import run_bass_kernel_spmd

    nc = _build_program()
    return run_bass_kernel_spmd(nc, in_maps, core_ids=list(range(_NCORES)), **kwargs)


def kernel(**inputs):
    res = _run(_shard_inputs(inputs["x1"], inputs["x2"]))
    return _gather(res.results)


# revision 12
# speedup vs baseline: 1.4052x; 1.4052x over previous
"""CostVolumeLayer3D Trainium2 kernel.

Computes cv[b, ch, d, y, x] = (1/125) * sum_c x1[b,c,d,y,x] * x2[b,c,d-h,y-i,x-j]
for the 45 channels that survive the reference's channel-collapse
(ch = (5*(i+j)+h) % 125, last write in (i,j,h) loop order wins -> for each
diagonal s=i+j the winner is i=min(2,s+2), j=s-i). Remaining 80 channels are 0.

Sharding: depth D=32 split across 8 cores (4 output slices each); the host
supplies each core a zero/halo-padded x2 shard so every shifted window is a
plain strided view.

Per-core layout: SBUF partitions = (b, c) = 2*64 = 128. Free axis = padded
(d', y', x') volume of x2, so a 3D shift is a free-axis offset view.
DVE computes shifted elementwise products (fp16, 2x mode); PE reduces over
the 64 channels via one-hot fp16 matmuls accumulating all 45 shifts into
PSUM rows 0..89 = (shift, b); ACT extracts with the 1/125 scale to fp32.
"""

import numpy as np

_B, _C, _D, _H, _W = 2, 64, 32, 64, 64
_R = 2
_NCH = 125
_NCORES = 8
_DL = _D // _NCORES          # output depth slices per core (4)
_DH = _DL + 2 * _R           # x2 depth slices incl. halo (8)
_YB = 32                     # y block
_NYB = _H // _YB             # 2
_YH = _YB + 2 * _R           # 36
_XH = _W + 2 * _R            # 68
_COMPUTE_DT = "float16"      # on-device product dtype
_NFREE = _YB * _W            # free elems per tile (2048)
_MMN = 512                   # matmul moving free dim


def _shift_table():
    """45 surviving shifts as (out_channel, d_off, y_off, x_off) where the
    x2 window for output (t, y, x) starts at padded index
    (t + d_off, y + y_off, x + x_off)."""
    shifts = []
    for sd in range(-4, 5):
        i = min(2, sd + 2)
        j = sd - i
        for h in range(-2, 3):
            shifts.append(((5 * sd + h) % _NCH, _R - h, _R - i, _R - j))
    return shifts


_SHIFTS = _shift_table()
_NS = len(_SHIFTS)           # 45
_M = 2 * _NS                 # psum rows: (shift, b)


def _ones_lhst(np_dt):
    """One matmul weight matrix per shift: lhsT[k, s, m] routes the partition
    half k//64 (= batch) of shift s's products to psum row 2*s + k//64."""
    a = np.zeros((128, _NS, _M), dtype=np_dt)
    for s in range(_NS):
        a[0:64, s, 2 * s] = 1.0
        a[64:128, s, 2 * s + 1] = 1.0
    return a


_prog = None


def _build_program():
    global _prog
    if _prog is not None:
        return _prog
    from contextlib import ExitStack

    import concourse.bacc as bacc
    import concourse.mybir as mybir
    import concourse.tile as tile

    dt_in = getattr(mybir.dt, _COMPUTE_DT)
    f32 = mybir.dt.float32
    nc = bacc.Bacc(trn_type="TRN2", debug=False)
    x1_d = nc.dram_tensor("x1", [_B, _C, _DL, _H, _W], dt_in, kind="ExternalInput")
    x2_d = nc.dram_tensor(
        "x2", [_B, _C, _DH, _H + 2 * _R, _XH], dt_in, kind="ExternalInput"
    )
    x2o_d = nc.dram_tensor(
        "x2o", [_B, _C, _DH, _H + 2 * _R, _XH], dt_in, kind="ExternalInput"
    )
    on_d = nc.dram_tensor("ones", [128, _NS, _M], dt_in, kind="ExternalInput")
    out_d = nc.dram_tensor("out", [_NS, _B, _DL, _H, _W], f32, kind="ExternalOutput")

    with tile.TileContext(nc) as tc:
        with ExitStack() as ctx:
            constp = ctx.enter_context(tc.tile_pool(name="const", bufs=1))
            x2p = ctx.enter_context(tc.tile_pool(name="x2res", bufs=1))
            x2op = ctx.enter_context(tc.tile_pool(name="x2odd", bufs=1))
            x1p = ctx.enter_context(tc.tile_pool(name="x1", bufs=3))
            prodp = ctx.enter_context(tc.tile_pool(name="prod", bufs=4))
            psump = ctx.enter_context(tc.tile_pool(name="psum", bufs=2, space="PSUM"))
            stagep = ctx.enter_context(tc.tile_pool(name="stage", bufs=2))

            ones_t = constp.tile([128, _NS, _M], dt_in)
            nc.sync.dma_start(ones_t[:], on_d.ap())

            for yh in range(_NYB):
                y0 = yh * _YB
                # x2o is x2 pre-shifted one x-element (host-built) so odd-j
                # windows stay 4B-aligned for the DVE 2x mode. Loads split
                # along d' to spread queues and shorten the critical head.
                x2_t = x2p.tile([128, _DH, _YH, _XH], dt_in)
                x2o_t = x2op.tile([128, _DH, _YH, _XH], dt_in)
                for dlo in range(0, _DH, 2):
                    for dram, tile_ in ((x2_d, x2_t), (x2o_d, x2o_t)):
                        nc.sync.dma_start(
                            tile_[:, dlo : dlo + 2],
                            dram.ap()[:, :, dlo : dlo + 2, y0 : y0 + _YH, :].rearrange(
                                "b c d y x -> (b c) d y x"
                            ),
                        )
                for t in range(_DL):
                    x1_t = x1p.tile([128, _YB, _W], dt_in)
                    nc.sync.dma_start(
                        x1_t[:],
                        x1_d.ap()[:, :, t, y0 : y0 + _YB, :].rearrange(
                            "b c y x -> (b c) y x"
                        ),
                    )
                    ps = psump.tile([128, _NFREE], f32)
                    for s, (_ch, dd0, yy0, xx0) in enumerate(_SHIFTS):
                        pr = prodp.tile([128, _YB, _W], dt_in)
                        dv = t + dd0
                        if xx0 % 2 == 0:
                            xv = x2_t[:, dv, yy0 : yy0 + _YB, xx0 : xx0 + _W]
                        else:
                            xv = x2o_t[:, dv, yy0 : yy0 + _YB, xx0 - 1 : xx0 - 1 + _W]
                        nc.vector.tensor_mul(pr[:], x1_t[:], xv)
                        prf = pr[:].rearrange("p y x -> p (y x)")
                        for n in range(_NFREE // _MMN):
                            nc.tensor.matmul(
                                ps[0:_M, _MMN * n : _MMN * (n + 1)],
                                lhsT=ones_t[:, s, :],
                                rhs=prf[:, _MMN * n : _MMN * (n + 1)],
                                start=(s == 0),
                                stop=(s == _NS - 1),
                            )
                    st = stagep.tile([128, _NFREE], f32)
                    nc.scalar.mul(st[0:_M, :], ps[0:_M, :], 1.0 / _NCH)
                    nc.sync.dma_start(
                        out_d.ap()[:, :, t, y0 : y0 + _YB, :].rearrange(
                            "s b y x -> (s b) (y x)"
                        ),
                        st[0:_M, :],
                    )
    nc.compile()
    _prog = nc
    return nc


def _np_dt():
    return np.float16 if _COMPUTE_DT == "float16" else np.float32


def _shard_inputs(x1, x2):
    np_dt = _np_dt()
    x2pad = np.pad(
        np.asarray(x2), ((0, 0), (0, 0), (_R, _R), (_R, _R), (_R, _R))
    ).astype(np_dt)
    x2odd = np.zeros_like(x2pad)
    x2odd[..., :-1] = x2pad[..., 1:]  # x2odd[x] = x2pad[x+1]
    x1 = np.asarray(x1)
    ones_np = _ones_lhst(np_dt)
    in_maps = []
    for k in range(_NCORES):
        d0 = k * _DL
        in_maps.append(
            {
                "x1": np.ascontiguousarray(x1[:, :, d0 : d0 + _DL].astype(np_dt)),
                "x2": np.ascontiguousarray(x2pad[:, :, d0 : d0 + _DH]),
                "x2o": np.ascontiguousarray(x2odd[:, :, d0 : d0 + _DH]),
                "ones": ones_np,
            }
        )
    return in_maps


def _gather(results):
    out = np.zeros((_B, _NCH, _D, _H, _W), dtype=np.float32)
    for k in range(_NCORES):
        o = results[k]["out"]  # [45, B, DL, H, W] fp32
        d0 = k * _DL
        for s, (ch, _dd0, _yy0, _xx0) in enumerate(_SHIFTS):
            out[:, ch, d0 : d0 + _DL] = o[s]
    return out


def _run(in_maps, **kwargs):
    from concourse.bass_utils import run_bass_kernel_spmd

    nc = _build_program()
    return run_bass_kernel_spmd(nc, in_maps, core_ids=list(range(_NCORES)), **kwargs)


def kernel(**inputs):
    res = _run(_shard_inputs(inputs["x1"], inputs["x2"]))
    return _gather(res.results)


# revision 17
# speedup vs baseline: 1.4757x; 1.0502x over previous
"""CostVolumeLayer3D Trainium2 kernel.

Computes cv[b, ch, d, y, x] = (1/125) * sum_c x1[b,c,d,y,x] * x2[b,c,d-h,y-i,x-j]
for the 45 channels that survive the reference's channel-collapse
(ch = (5*(i+j)+h) % 125, last write in (i,j,h) loop order wins -> for each
diagonal s=i+j the winner is i=min(2,s+2), j=s-i). Remaining 80 channels are 0.

Sharding: depth D=32 split across 8 cores (4 output slices each); the host
supplies each core a zero/halo-padded x2 shard so every shifted window is a
plain strided view.

Per-core layout: SBUF partitions = (b, c) = 2*64 = 128. Free axis = padded
(d', y', x') volume of x2, so a 3D shift is a free-axis offset view.
DVE computes shifted elementwise products (fp16, 2x mode); PE reduces over
the 64 channels via one-hot fp16 matmuls accumulating all 45 shifts into
PSUM rows 0..89 = (shift, b); ACT extracts with the 1/125 scale to fp32.
"""

import numpy as np

_B, _C, _D, _H, _W = 2, 64, 32, 64, 64
_R = 2
_NCH = 125
_RNG = 2 * _R + 1            # window extent per axis (5)
_NCORES = 8
_DL = _D // _NCORES          # output depth slices per core (4)
_DH = _DL + 2 * _R           # x2 depth slices incl. halo (8)
_YB = 32                     # y block
_NYB = _H // _YB             # 2
_YH = _YB + 2 * _R           # 36
_XH = _W + 2 * _R            # 68
_COMPUTE_DT = "float16"      # on-device product dtype
_NFREE = _YB * _W            # free elems per tile (2048)
_MMN = 512                   # matmul moving free dim


def _shift_table():
    """45 surviving shifts as (out_channel, d_off, y_off, x_off) where the
    x2 window for output (t, y, x) starts at padded index
    (t + d_off, y + y_off, x + x_off)."""
    shifts = []
    for sd in range(-4, 5):
        i = min(2, sd + 2)
        j = sd - i
        for h in range(-2, 3):
            shifts.append(((5 * sd + h) % _NCH, _R - h, _R - i, _R - j))
    return shifts


_SHIFTS = _shift_table()
_NS = len(_SHIFTS)           # 45
_M = 2 * _NS                 # psum rows: (shift, b)


def _ones_lhst(np_dt):
    """One matmul weight matrix per shift: lhsT[k, s, m] routes the partition
    half k//64 (= batch) of shift s's products to psum row 2*s + k//64."""
    a = np.zeros((128, _NS, _M), dtype=np_dt)
    for s in range(_NS):
        a[0:64, s, 2 * s] = 1.0
        a[64:128, s, 2 * s + 1] = 1.0
    return a


_prog = None


def _build_program():
    global _prog
    if _prog is not None:
        return _prog
    from contextlib import ExitStack

    import concourse.bacc as bacc
    import concourse.mybir as mybir
    import concourse.tile as tile

    dt_in = getattr(mybir.dt, _COMPUTE_DT)
    f32 = mybir.dt.float32
    nc = bacc.Bacc(trn_type="TRN2", debug=False)
    x1_d = nc.dram_tensor("x1", [_B, _C, _DL, _H, _W], dt_in, kind="ExternalInput")
    x2_d = nc.dram_tensor(
        "x2", [_B, _C, _DH, _H + 2 * _R, _XH], dt_in, kind="ExternalInput"
    )
    on_d = nc.dram_tensor("ones", [128, _NS, _M], dt_in, kind="ExternalInput")
    out_d = nc.dram_tensor("out", [_NS, _B, _DL, _H, _W], f32, kind="ExternalOutput")

    with tile.TileContext(nc) as tc:
        with ExitStack() as ctx:
            constp = ctx.enter_context(tc.tile_pool(name="const", bufs=1))
            x2p = ctx.enter_context(tc.tile_pool(name="x2res", bufs=1))
            x2op = ctx.enter_context(tc.tile_pool(name="x2odd", bufs=1))
            x1p = ctx.enter_context(tc.tile_pool(name="x1", bufs=3))
            prodp = ctx.enter_context(tc.tile_pool(name="prod", bufs=2))
            psump = ctx.enter_context(tc.tile_pool(name="psum", bufs=2, space="PSUM"))
            stagep = ctx.enter_context(tc.tile_pool(name="stage", bufs=2))

            ones_t = constp.tile([128, _NS, _M], dt_in)
            nc.sync.dma_start(ones_t[:], on_d.ap())

            for yh in range(_NYB):
                y0 = yh * _YB
                # x2 loads split along d' to spread DMA queues; behind each
                # chunk an ACT copy builds the one-x-element-shifted twin so
                # odd-j windows stay 4B-aligned for the DVE 2x mode.
                x2_t = x2p.tile([128, _DH, _YH, _XH], dt_in)
                x2o_t = x2op.tile([128, _DH, _YH, _XH], dt_in)
                for dlo in range(0, _DH, 2):
                    nc.sync.dma_start(
                        x2_t[:, dlo : dlo + 2],
                        x2_d.ap()[:, :, dlo : dlo + 2, y0 : y0 + _YH, :].rearrange(
                            "b c d y x -> (b c) d y x"
                        ),
                    )
                    nc.scalar.copy(
                        x2o_t[:, dlo : dlo + 2, :, 0 : _XH - 1],
                        x2_t[:, dlo : dlo + 2, :, 1:_XH],
                    )
                for t in range(_DL):
                    x1_t = x1p.tile([128, _YB, _W], dt_in)
                    nc.sync.dma_start(
                        x1_t[:],
                        x1_d.ap()[:, :, t, y0 : y0 + _YB, :].rearrange(
                            "b c y x -> (b c) y x"
                        ),
                    )
                    x1_b = x1_t[:].unsqueeze(1).broadcast_to([128, _RNG, _YB, _W])
                    ps = psump.tile([128, _NFREE], f32)
                    # one TT per diagonal sd: its 5 h-shifts are an arithmetic
                    # d'-progression, so a single strided 5x-wide op covers them
                    for di in range(_NS // _RNG):
                        _ch, _dd0, yy0, xx0 = _SHIFTS[_RNG * di]
                        if xx0 % 2 == 0:
                            xv = x2_t[:, t : t + _RNG, yy0 : yy0 + _YB, xx0 : xx0 + _W]
                        else:
                            xv = x2o_t[
                                :, t : t + _RNG, yy0 : yy0 + _YB, xx0 - 1 : xx0 - 1 + _W
                            ]
                        pr = prodp.tile([128, _RNG, _YB, _W], dt_in)
                        nc.vector.tensor_mul(pr[:], x1_b, xv)
                        prf = pr[:].rearrange("p h y x -> p h (y x)")
                        for q in range(_RNG):
                            # pr[:, q] is the shift with dd0 == q, i.e. h = 2-q
                            s = _RNG * di + (_RNG - 1 - q)
                            for n in range(_NFREE // _MMN):
                                nc.tensor.matmul(
                                    ps[0:_M, _MMN * n : _MMN * (n + 1)],
                                    lhsT=ones_t[:, s, :],
                                    rhs=prf[:, q, _MMN * n : _MMN * (n + 1)],
                                    start=(di == 0 and q == 0),
                                    stop=(di == _NS // _RNG - 1 and q == _RNG - 1),
                                )
                    st = stagep.tile([128, _NFREE], f32)
                    nc.scalar.mul(st[0:_M, :], ps[0:_M, :], 1.0 / _NCH)
                    nc.sync.dma_start(
                        out_d.ap()[:, :, t, y0 : y0 + _YB, :].rearrange(
                            "s b y x -> (s b) (y x)"
                        ),
                        st[0:_M, :],
                    )
    nc.compile()
    _prog = nc
    return nc


def _np_dt():
    return np.float16 if _COMPUTE_DT == "float16" else np.float32


def _shard_inputs(x1, x2):
    np_dt = _np_dt()
    x2pad = np.pad(
        np.asarray(x2), ((0, 0), (0, 0), (_R, _R), (_R, _R), (_R, _R))
    ).astype(np_dt)
    x1 = np.asarray(x1)
    ones_np = _ones_lhst(np_dt)
    in_maps = []
    for k in range(_NCORES):
        d0 = k * _DL
        in_maps.append(
            {
                "x1": np.ascontiguousarray(x1[:, :, d0 : d0 + _DL].astype(np_dt)),
                "x2": np.ascontiguousarray(x2pad[:, :, d0 : d0 + _DH]),
                "ones": ones_np,
            }
        )
    return in_maps


def _gather(results):
    out = np.zeros((_B, _NCH, _D, _H, _W), dtype=np.float32)
    for k in range(_NCORES):
        o = results[k]["out"]  # [45, B, DL, H, W] fp32
        d0 = k * _DL
        for s, (ch, _dd0, _yy0, _xx0) in enumerate(_SHIFTS):
            out[:, ch, d0 : d0 + _DL] = o[s]
    return out


def _run(in_maps, **kwargs):
    from concourse.bass_utils import run_bass_kernel_spmd

    nc = _build_program()
    return run_bass_kernel_spmd(nc, in_maps, core_ids=list(range(_NCORES)), **kwargs)


def kernel(**inputs):
    res = _run(_shard_inputs(inputs["x1"], inputs["x2"]))
    return _gather(res.results)


# revision 19
# speedup vs baseline: 1.5648x; 1.0604x over previous
"""CostVolumeLayer3D Trainium2 kernel.

Computes cv[b, ch, d, y, x] = (1/125) * sum_c x1[b,c,d,y,x] * x2[b,c,d-h,y-i,x-j]
for the 45 channels that survive the reference's channel-collapse
(ch = (5*(i+j)+h) % 125, last write in (i,j,h) loop order wins -> for each
diagonal s=i+j the winner is i=min(2,s+2), j=s-i). Remaining 80 channels are 0.

Sharding: depth D=32 split across 8 cores (4 output slices each); the host
supplies each core a zero/halo-padded x2 shard so every shifted window is a
plain strided view.

Per-core layout: SBUF partitions = (b, c) = 2*64 = 128. Free axis = padded
(d', y', x') volume of x2, so a 3D shift is a free-axis offset view.
DVE computes shifted elementwise products (fp16, 2x mode); PE reduces over
the 64 channels via one-hot fp16 matmuls accumulating all 45 shifts into
PSUM rows 0..89 = (shift, b); ACT extracts with the 1/125 scale to fp32.
"""

import numpy as np

_B, _C, _D, _H, _W = 2, 64, 32, 64, 64
_R = 2
_NCH = 125
_RNG = 2 * _R + 1            # window extent per axis (5)
_NCORES = 8
_DL = _D // _NCORES          # output depth slices per core (4)
_DH = _DL + 2 * _R           # x2 depth slices incl. halo (8)
_YB = 16                     # y block
_NYB = _H // _YB             # 4
_YH = _YB + 2 * _R           # 36
_XH = _W + 2 * _R            # 68
_COMPUTE_DT = "float16"      # on-device product dtype
_NFREE = _YB * _W            # free elems per tile (2048)
_MMN = 512                   # matmul moving free dim


def _shift_table():
    """45 surviving shifts as (out_channel, d_off, y_off, x_off) where the
    x2 window for output (t, y, x) starts at padded index
    (t + d_off, y + y_off, x + x_off)."""
    shifts = []
    for sd in range(-4, 5):
        i = min(2, sd + 2)
        j = sd - i
        for h in range(-2, 3):
            shifts.append(((5 * sd + h) % _NCH, _R - h, _R - i, _R - j))
    return shifts


_SHIFTS = _shift_table()
_NS = len(_SHIFTS)           # 45
_M = 2 * _NS                 # psum rows: (shift, b)


def _ones_lhst(np_dt):
    """One matmul weight matrix per shift: lhsT[k, s, m] routes the partition
    half k//64 (= batch) of shift s's products to psum row 2*s + k//64."""
    a = np.zeros((128, _NS, _M), dtype=np_dt)
    for s in range(_NS):
        a[0:64, s, 2 * s] = 1.0
        a[64:128, s, 2 * s + 1] = 1.0
    return a


_prog = None


def _build_program():
    global _prog
    if _prog is not None:
        return _prog
    from contextlib import ExitStack

    import concourse.bacc as bacc
    import concourse.mybir as mybir
    import concourse.tile as tile

    dt_in = getattr(mybir.dt, _COMPUTE_DT)
    f32 = mybir.dt.float32
    nc = bacc.Bacc(trn_type="TRN2", debug=False)
    x1_d = nc.dram_tensor("x1", [_B, _C, _DL, _H, _W], dt_in, kind="ExternalInput")
    x2_d = nc.dram_tensor(
        "x2", [_B, _C, _DH, _H + 2 * _R, _XH], dt_in, kind="ExternalInput"
    )
    on_d = nc.dram_tensor("ones", [128, _NS, _M], dt_in, kind="ExternalInput")
    out_d = nc.dram_tensor("out", [_NS, _B, _DL, _H, _W], f32, kind="ExternalOutput")

    with tile.TileContext(nc) as tc:
        with ExitStack() as ctx:
            constp = ctx.enter_context(tc.tile_pool(name="const", bufs=1))
            x2p = ctx.enter_context(tc.tile_pool(name="x2res", bufs=2))
            x2op = ctx.enter_context(tc.tile_pool(name="x2odd", bufs=2))
            x1p = ctx.enter_context(tc.tile_pool(name="x1", bufs=3))
            prodp = ctx.enter_context(tc.tile_pool(name="prod", bufs=2))
            psump = ctx.enter_context(tc.tile_pool(name="psum", bufs=2, space="PSUM"))
            stagep = ctx.enter_context(tc.tile_pool(name="stage", bufs=2))

            ones_t = constp.tile([128, _NS, _M], dt_in)
            nc.sync.dma_start(ones_t[:], on_d.ap())

            for yh in range(_NYB):
                y0 = yh * _YB
                # x2 loads split along d' to spread DMA queues; behind each
                # chunk an ACT copy builds the one-x-element-shifted twin so
                # odd-j windows stay 4B-aligned for the DVE 2x mode.
                x2_t = x2p.tile([128, _DH, _YH, _XH], dt_in)
                x2o_t = x2op.tile([128, _DH, _YH, _XH], dt_in)
                for dlo in range(0, _DH, 2):
                    nc.sync.dma_start(
                        x2_t[:, dlo : dlo + 2],
                        x2_d.ap()[:, :, dlo : dlo + 2, y0 : y0 + _YH, :].rearrange(
                            "b c d y x -> (b c) d y x"
                        ),
                    )
                    nc.scalar.copy(
                        x2o_t[:, dlo : dlo + 2, :, 0 : _XH - 1],
                        x2_t[:, dlo : dlo + 2, :, 1:_XH],
                    )
                for t in range(_DL):
                    x1_t = x1p.tile([128, _YB, _W], dt_in)
                    nc.sync.dma_start(
                        x1_t[:],
                        x1_d.ap()[:, :, t, y0 : y0 + _YB, :].rearrange(
                            "b c y x -> (b c) y x"
                        ),
                    )
                    x1_b = x1_t[:].unsqueeze(1).broadcast_to([128, _RNG, _YB, _W])
                    ps = psump.tile([128, _NFREE], f32)
                    # one TT per diagonal sd: its 5 h-shifts are an arithmetic
                    # d'-progression, so a single strided 5x-wide op covers them
                    for di in range(_NS // _RNG):
                        _ch, _dd0, yy0, xx0 = _SHIFTS[_RNG * di]
                        if xx0 % 2 == 0:
                            xv = x2_t[:, t : t + _RNG, yy0 : yy0 + _YB, xx0 : xx0 + _W]
                        else:
                            xv = x2o_t[
                                :, t : t + _RNG, yy0 : yy0 + _YB, xx0 - 1 : xx0 - 1 + _W
                            ]
                        pr = prodp.tile([128, _RNG, _YB, _W], dt_in)
                        nc.vector.tensor_mul(pr[:], x1_b, xv)
                        prf = pr[:].rearrange("p h y x -> p h (y x)")
                        for q in range(_RNG):
                            # pr[:, q] is the shift with dd0 == q, i.e. h = 2-q
                            s = _RNG * di + (_RNG - 1 - q)
                            for n in range(_NFREE // _MMN):
                                nc.tensor.matmul(
                                    ps[0:_M, _MMN * n : _MMN * (n + 1)],
                                    lhsT=ones_t[:, s, :],
                                    rhs=prf[:, q, _MMN * n : _MMN * (n + 1)],
                                    start=(di == 0 and q == 0),
                                    stop=(di == _NS // _RNG - 1 and q == _RNG - 1),
                                )
                    st = stagep.tile([128, _NFREE], f32)
                    nc.scalar.mul(st[0:_M, :], ps[0:_M, :], 1.0 / _NCH)
                    nc.sync.dma_start(
                        out_d.ap()[:, :, t, y0 : y0 + _YB, :].rearrange(
                            "s b y x -> (s b) (y x)"
                        ),
                        st[0:_M, :],
                    )
    nc.compile()
    _prog = nc
    return nc


def _np_dt():
    return np.float16 if _COMPUTE_DT == "float16" else np.float32


def _shard_inputs(x1, x2):
    np_dt = _np_dt()
    x2pad = np.pad(
        np.asarray(x2), ((0, 0), (0, 0), (_R, _R), (_R, _R), (_R, _R))
    ).astype(np_dt)
    x1 = np.asarray(x1)
    ones_np = _ones_lhst(np_dt)
    in_maps = []
    for k in range(_NCORES):
        d0 = k * _DL
        in_maps.append(
            {
                "x1": np.ascontiguousarray(x1[:, :, d0 : d0 + _DL].astype(np_dt)),
                "x2": np.ascontiguousarray(x2pad[:, :, d0 : d0 + _DH]),
                "ones": ones_np,
            }
        )
    return in_maps


def _gather(results):
    out = np.zeros((_B, _NCH, _D, _H, _W), dtype=np.float32)
    for k in range(_NCORES):
        o = results[k]["out"]  # [45, B, DL, H, W] fp32
        d0 = k * _DL
        for s, (ch, _dd0, _yy0, _xx0) in enumerate(_SHIFTS):
            out[:, ch, d0 : d0 + _DL] = o[s]
    return out


def _run(in_maps, **kwargs):
    from concourse.bass_utils import run_bass_kernel_spmd

    nc = _build_program()
    return run_bass_kernel_spmd(nc, in_maps, core_ids=list(range(_NCORES)), **kwargs)


def kernel(**inputs):
    res = _run(_shard_inputs(inputs["x1"], inputs["x2"]))
    return _gather(res.results)


# revision 24
# speedup vs baseline: 1.5677x; 1.0019x over previous
"""CostVolumeLayer3D Trainium2 kernel.

Computes cv[b, ch, d, y, x] = (1/125) * sum_c x1[b,c,d,y,x] * x2[b,c,d-h,y-i,x-j]
for the 45 channels that survive the reference's channel-collapse
(ch = (5*(i+j)+h) % 125, last write in (i,j,h) loop order wins -> for each
diagonal s=i+j the winner is i=min(2,s+2), j=s-i). Remaining 80 channels are 0.

Sharding: depth D=32 split across 8 cores (4 output slices each); the host
supplies each core a zero/halo-padded x2 shard so every shifted window is a
plain strided view.

Per-core layout: SBUF partitions = (b, c) = 2*64 = 128. Free axis = padded
(d', y', x') volume of x2, so a 3D shift is a free-axis offset view.
DVE computes shifted elementwise products (fp16, 2x mode); PE reduces over
the 64 channels via one-hot fp16 matmuls accumulating all 45 shifts into
PSUM rows 0..89 = (shift, b); ACT extracts with the 1/125 scale to fp32.
"""

import numpy as np

_B, _C, _D, _H, _W = 2, 64, 32, 64, 64
_R = 2
_NCH = 125
_RNG = 2 * _R + 1            # window extent per axis (5)
_NCORES = 8
_DL = _D // _NCORES          # output depth slices per core (4)
_DH = _DL + 2 * _R           # x2 depth slices incl. halo (8)
_YBLOCKS = ((0, 8), (8, 8), (16, 16), (32, 16), (48, 16))  # (y0, rows)
_XH = _W + 2 * _R            # 68
_COMPUTE_DT = "float16"      # on-device product dtype
_MMN = 512                   # matmul moving free dim


def _shift_table():
    """45 surviving shifts as (out_channel, d_off, y_off, x_off) where the
    x2 window for output (t, y, x) starts at padded index
    (t + d_off, y + y_off, x + x_off)."""
    shifts = []
    for sd in range(-4, 5):
        i = min(2, sd + 2)
        j = sd - i
        for h in range(-2, 3):
            shifts.append(((5 * sd + h) % _NCH, _R - h, _R - i, _R - j))
    return shifts


_SHIFTS = _shift_table()
_NS = len(_SHIFTS)           # 45
_M = 2 * _NS                 # psum rows: (shift, b)


def _ones_lhst(np_dt):
    """One matmul weight matrix per shift: lhsT[k, s, m] routes the partition
    half k//64 (= batch) of shift s's products to psum row 2*s + k//64."""
    a = np.zeros((128, _NS, _M), dtype=np_dt)
    for s in range(_NS):
        a[0:64, s, 2 * s] = 1.0
        a[64:128, s, 2 * s + 1] = 1.0
    return a


_prog = None


def _build_program():
    global _prog
    if _prog is not None:
        return _prog
    from contextlib import ExitStack

    import concourse.bacc as bacc
    import concourse.mybir as mybir
    import concourse.tile as tile

    dt_in = getattr(mybir.dt, _COMPUTE_DT)
    f32 = mybir.dt.float32
    nc = bacc.Bacc(trn_type="TRN2", debug=False)
    x1_d = nc.dram_tensor("x1", [_B, _C, _DL, _H, _W], dt_in, kind="ExternalInput")
    x2_d = nc.dram_tensor(
        "x2", [_B, _C, _DH, _H + 2 * _R, _XH], dt_in, kind="ExternalInput"
    )
    on_d = nc.dram_tensor("ones", [128, _NS, _M], dt_in, kind="ExternalInput")
    out_d = nc.dram_tensor("out", [_NS, _B, _DL, _H, _W], f32, kind="ExternalOutput")

    with tile.TileContext(nc) as tc:
        with ExitStack() as ctx:
            constp = ctx.enter_context(tc.tile_pool(name="const", bufs=1))
            x2p = ctx.enter_context(tc.tile_pool(name="x2res", bufs=2))
            x2op = ctx.enter_context(tc.tile_pool(name="x2odd", bufs=2))
            x1p = ctx.enter_context(tc.tile_pool(name="x1", bufs=4))
            prodp = ctx.enter_context(tc.tile_pool(name="prod", bufs=3))
            psump = ctx.enter_context(tc.tile_pool(name="psum", bufs=4, space="PSUM"))
            stagep = ctx.enter_context(tc.tile_pool(name="stage", bufs=3))

            ones_t = constp.tile([128, _NS, _M], dt_in)
            nc.sync.dma_start(ones_t[:], on_d.ap())

            for y0, yb in _YBLOCKS:
                yhh = yb + 2 * _R
                nfree = yb * _W
                # x2 loads split along d' to spread DMA queues; behind each
                # chunk an ACT copy builds the one-x-element-shifted twin so
                # odd-j windows stay 4B-aligned for the DVE 2x mode.
                x2_t = x2p.tile([128, _DH, yhh, _XH], dt_in, tag="x2res")
                x2o_t = x2op.tile([128, _DH, yhh, _XH], dt_in, tag="x2odd")
                for dlo in range(0, _DH, 2):
                    nc.sync.dma_start(
                        x2_t[:, dlo : dlo + 2],
                        x2_d.ap()[:, :, dlo : dlo + 2, y0 : y0 + yhh, :].rearrange(
                            "b c d y x -> (b c) d y x"
                        ),
                    )
                    nc.scalar.copy(
                        x2o_t[:, dlo : dlo + 2, :, 0 : _XH - 1],
                        x2_t[:, dlo : dlo + 2, :, 1:_XH],
                    )
                for t in range(_DL):
                    x1_t = x1p.tile([128, yb, _W], dt_in, tag="x1")
                    nc.sync.dma_start(
                        x1_t[:],
                        x1_d.ap()[:, :, t, y0 : y0 + yb, :].rearrange(
                            "b c y x -> (b c) y x"
                        ),
                    )
                    x1_b = x1_t[:].unsqueeze(1).broadcast_to([128, _RNG, yb, _W])
                    ps = psump.tile([128, nfree], f32, tag="ps")
                    # one TT per diagonal sd: its 5 h-shifts are an arithmetic
                    # d'-progression, so a single strided 5x-wide op covers them
                    for di in range(_NS // _RNG):
                        _ch, _dd0, yy0, xx0 = _SHIFTS[_RNG * di]
                        if xx0 % 2 == 0:
                            xv = x2_t[:, t : t + _RNG, yy0 : yy0 + yb, xx0 : xx0 + _W]
                        else:
                            xv = x2o_t[
                                :, t : t + _RNG, yy0 : yy0 + yb, xx0 - 1 : xx0 - 1 + _W
                            ]
                        pr = prodp.tile([128, _RNG, yb, _W], dt_in, tag="pr")
                        nc.vector.tensor_mul(pr[:], x1_b, xv)
                        prf = pr[:].rearrange("p h y x -> p h (y x)")
                        for q in range(_RNG):
                            # pr[:, q] is the shift with dd0 == q, i.e. h = 2-q
                            s = _RNG * di + (_RNG - 1 - q)
                            for n in range(nfree // _MMN):
                                nc.tensor.matmul(
                                    ps[0:_M, _MMN * n : _MMN * (n + 1)],
                                    lhsT=ones_t[:, s, :],
                                    rhs=prf[:, q, _MMN * n : _MMN * (n + 1)],
                                    start=(di == 0 and q == 0),
                                    stop=(di == _NS // _RNG - 1 and q == _RNG - 1),
                                )
                    st = stagep.tile([128, nfree], f32, tag="st")
                    nc.scalar.mul(st[0:_M, :], ps[0:_M, :], 1.0 / _NCH)
                    nc.sync.dma_start(
                        out_d.ap()[:, :, t, y0 : y0 + yb, :].rearrange(
                            "s b y x -> (s b) (y x)"
                        ),
                        st[0:_M, :],
                    )
    nc.compile()
    _prog = nc
    return nc


def _np_dt():
    return np.float16 if _COMPUTE_DT == "float16" else np.float32


def _shard_inputs(x1, x2):
    np_dt = _np_dt()
    x2pad = np.pad(
        np.asarray(x2), ((0, 0), (0, 0), (_R, _R), (_R, _R), (_R, _R))
    ).astype(np_dt)
    x1 = np.asarray(x1)
    ones_np = _ones_lhst(np_dt)
    in_maps = []
    for k in range(_NCORES):
        d0 = k * _DL
        in_maps.append(
            {
                "x1": np.ascontiguousarray(x1[:, :, d0 : d0 + _DL].astype(np_dt)),
                "x2": np.ascontiguousarray(x2pad[:, :, d0 : d0 + _DH]),
                "ones": ones_np,
            }
        )
    return in_maps


def _gather(results):
    out = np.zeros((_B, _NCH, _D, _H, _W), dtype=np.float32)
    for k in range(_NCORES):
        o = results[k]["out"]  # [45, B, DL, H, W] fp32
        d0 = k * _DL
        for s, (ch, _dd0, _yy0, _xx0) in enumerate(_SHIFTS):
            out[:, ch, d0 : d0 + _DL] = o[s]
    return out


def _run(in_maps, **kwargs):
    from concourse.bass_utils import run_bass_kernel_spmd

    nc = _build_program()
    return run_bass_kernel_spmd(nc, in_maps, core_ids=list(range(_NCORES)), **kwargs)


def kernel(**inputs):
    res = _run(_shard_inputs(inputs["x1"], inputs["x2"]))
    return _gather(res.results)
